# revision 1
# baseline (speedup 1.0000x reference)
"""MinibatchDiscrimination TRN2 kernel.

x: [512, 1024] f32, T: [1024, 1024] f32.
M = (x @ T).reshape(512, 64, 16); l1[i,j,k] = sum_d |M[i,k,d]-M[j,k,d]|
out[i,k] = sum_j exp(-l1[i,j,k]) - 1.

Sharding: batch rows split across 8 cores (64 each), no collectives. Each
core's copy of x^T has its j-columns rolled so its own 64 rows sit at local
columns 0..63.

Symmetric pair coverage: core-local row i computes the j-window
[i+1, i+257) (pair distance d in [1, 256]). Over all cores/rows every
unordered pair {a, a+d} with d in [1, 255] is computed exactly once (its
exp(-l1) is accumulated both to row a via the activation accum_out and to
row a+d via a transposed column accumulator), while d = 256 pairs are
computed twice (once from each end) and accumulated row-side only. The
diagonal is never computed, so no -1 correction is needed.

|z| via relu: DVE tiles use relu(z) = (z sub m) max 0 (s=+1), ACT tiles use
Relu(-z) (s=-1); l1 = 2*sum relu - s*(G[k,j] - G[k,i]) with
G[k,j] = sum_d M[j,kd]. The -s*G[k,j] term is injected into the PSUM
accumulation via one extra matmul (lhsT=I64, rhs=Gsneg slice) and the
+s*G[k,i] term rides the exp's per-partition bias (bias = Gsneg[:, i]).

f32r everywhere on the PE (full-rate; fp32 matmul is 1/4 rate). The BIR
verifier requires f32r matmul operands to be produced pre-rounded, so DMA'd
fp32 data is rounded on-chip via DVE copies into f32r tiles.
"""

import contextlib

import numpy as np

import concourse.bass as bass
import concourse.tile as tile
from concourse import mybir
from concourse import bass_utils

B = 512
F = 1024
KD = 1024  # = NUM_KERNELS(64) * KERNEL_DIM(16)
NK = 64
N_CORES = 8
NI = B // N_CORES  # local rows per core
NT = KD // 128  # kd tiles
NF = F // 128  # f chunks
W = 256  # j-window width per row
JL = NI + W  # used local-j extent
LAG = 6  # transpose-add lag (iterations) to keep DVE from stalling on exp
# relu-tile engine split: t < N_DVE on VectorE (s=+1), rest on ScalarE (s=-1)
N_DVE = 5

_FP32 = mybir.dt.float32
_F32R = mybir.dt.float32r


def _split_all_waits(nc):
    """walrus in this env encodes at most 1 sync wait per instruction: hoist
    extra waits onto same-engine NOPs inserted just before the instruction.
    Safe because waits are AND-ed stall conditions on the engine's sequencer
    and semaphores are monotonic."""
    count = 0
    for fn in nc.m.functions:
        for bb in fn.blocks:
            insts = list(bb.instructions)
            new = []
            changed = False
            for inst in insts:
                si = getattr(inst, "sync_info", None)
                waits = list(si.on_wait) if (si is not None and si.on_wait) else []
                if len(waits) > 1:
                    for w in waits[:-1]:
                        nop = mybir.InstNoOp(name=f"NOPW-{count}", ins=[], outs=[])
                        count += 1
                        nop.engine = inst.engine
                        nop.sync_info = mybir.SyncInfo(on_wait=[w], on_update=[])
                        nc.register_instruction(nop, overwrite=True)
                        new.append(nop)
                    si.on_wait = [waits[-1]]
                    changed = True
                new.append(inst)
            if changed:
                bb.instructions[:] = new


def _patch_drain_wait_limit():
    if getattr(tile.TileContext, "_wait_split_patched", False):
        return
    orig = tile.TileContext.schedule_and_allocate

    def schedule_and_allocate(self, *a, **k):
        r = orig(self, *a, **k)
        _split_all_waits(self.nc)
        return r

    tile.TileContext.schedule_and_allocate = schedule_and_allocate
    tile.TileContext._wait_split_patched = True


def build_s_matrices():
    """One-hot d-sum matrices per kd-tile t (tile t holds k in [8t, 8t+8)):
    base S_t[p, m] = 1 iff m == t*8 + p//16. S2 = 2*S_t (relu matmuls),
    Sg = -s_t*S_t (builds Gsneg = -s*G), I64 = identity (G injection)."""
    S2 = np.zeros((128, NT * NK), dtype=np.float32)
    Sg = np.zeros((128, NT * NK), dtype=np.float32)
    for t in range(NT):
        s_t = 1.0 if t < N_DVE else -1.0
        for p in range(128):
            m = t * NK + t * 8 + p // 16
            S2[p, m] = 2.0
            Sg[p, m] = -s_t
    I64 = np.eye(NK, dtype=np.float32)
    return S2, Sg, I64


def build_program(repeat: int = 1):
    _patch_drain_wait_limit()
    nc = bass.Bass(
        "TRN2", target_bir_lowering=False, debug=False, num_devices=N_CORES
    )
    xT_d = nc.dram_tensor("xT", [F, JL], _FP32, kind="ExternalInput").ap()
    T_d = nc.dram_tensor("T", [F, KD], _FP32, kind="ExternalInput").ap()
    S2_d = nc.dram_tensor("S2", [128, NT * NK], _FP32, kind="ExternalInput").ap()
    Sg_d = nc.dram_tensor("Sg", [128, NT * NK], _FP32, kind="ExternalInput").ap()
    I64_d = nc.dram_tensor("I64", [NK, NK], _FP32, kind="ExternalInput").ap()
    orow_d = nc.dram_tensor("orow", [NK, NI], _FP32, kind="ExternalOutput").ap()
    ocol_d = nc.dram_tensor("ocol", [NK, JL], _FP32, kind="ExternalOutput").ap()

    AF = mybir.ActivationFunctionType
    AO = mybir.AluOpType

    with tile.TileContext(nc) as tc:
        with (
            tc.tile_pool(name="stage", bufs=3) as stage,
            tc.tile_pool(name="tr", bufs=NF) as tr_pool,
            tc.tile_pool(name="xr", bufs=NF) as xr_pool,
            tc.tile_pool(name="mt", bufs=NT) as mt_pool,
            tc.tile_pool(name="ssb", bufs=1) as s_pool,
            tc.tile_pool(name="relu", bufs=16) as relu_pool,
            tc.tile_pool(name="ep", bufs=LAG + 3) as e_pool,
            tc.tile_pool(name="op", bufs=1) as o_pool,
            tc.tile_pool(name="pmm", bufs=2, space="PSUM") as psum_mm,
            tc.tile_pool(name="pl1", bufs=6, space="PSUM") as psum_l1,
            tc.For_i(0, repeat, 1) if repeat > 1 else contextlib.nullcontext(),
        ):
            # ---- loads + f32r rounding ----
            T_r = []
            for f in range(NF):
                st = stage.tile([128, KD], _FP32, tag="stage")
                nc.sync.dma_start(out=st, in_=T_d[f * 128 : (f + 1) * 128, :])
                tr = tr_pool.tile([128, KD], _F32R, tag="tr")
                nc.vector.tensor_copy(tr, st)
                T_r.append(tr)
            x_r = []
            for f in range(NF):
                st = stage.tile([128, JL], _FP32, tag="xstage")
                nc.sync.dma_start(out=st, in_=xT_d[f * 128 : (f + 1) * 128, :])
                xr = xr_pool.tile([128, JL], _F32R, tag="xr")
                nc.vector.tensor_copy(xr, st)
                x_r.append(xr)
            st = stage.tile([128, NT * NK], _FP32, tag="s2stage")
            nc.sync.dma_start(out=st, in_=S2_d)
            S2_r = s_pool.tile([128, NT * NK], _F32R, tag="s2r")
            nc.vector.tensor_copy(S2_r, st)
            st = stage.tile([128, NT * NK], _FP32, tag="sgstage")
            nc.sync.dma_start(out=st, in_=Sg_d)
            Sg_r = s_pool.tile([128, NT * NK], _F32R, tag="sgr")
            nc.vector.tensor_copy(Sg_r, st)
            st = stage.tile([NK, NK], _FP32, tag="i64stage")
            nc.sync.dma_start(out=st, in_=I64_d)
            I64_r = s_pool.tile([NK, NK], _F32R, tag="i64r")
            nc.vector.tensor_copy(I64_r, st)

            # ---- phase 1: M^T tiles [128 kd, JL j] (f32r) ----
            mt = []
            for t in range(NT):
                pm = psum_mm.tile([128, JL], _FP32, tag="pmm")
                for f in range(NF):
                    nc.tensor.matmul(
                        pm,
                        lhsT=T_r[f][:, t * 128 : (t + 1) * 128],
                        rhs=x_r[f],
                        start=(f == 0),
                        stop=(f == NF - 1),
                    )
                m = mt_pool.tile([128, JL], _F32R, tag="mt")
                if t % 2 == 0:
                    nc.vector.tensor_copy(m, pm)
                else:
                    nc.scalar.copy(m, pm)
                mt.append(m)

            # ---- phase 1.5: Gsneg[k, j] = -s_k * sum_d M[j, kd] ----
            pg = psum_mm.tile([NK, JL], _FP32, tag="pmm")
            for t in range(NT):
                nc.tensor.matmul(
                    pg,
                    lhsT=Sg_r[:, t * NK : (t + 1) * NK],
                    rhs=mt[t],
                    start=(t == 0),
                    stop=(t == NT - 1),
                )
            Gsneg = s_pool.tile([NK, JL], _F32R, tag="gsneg")
            nc.vector.tensor_copy(Gsneg, pg)

            O_row = o_pool.tile([NK, NI], _FP32, tag="orow")
            O_col = o_pool.tile([NK, JL], _FP32, tag="ocol")
            nc.vector.memset(O_col, 0.0)

            # ---- phase 2 ----
            e_tiles = [None] * NI

            def emit_col_add(j):
                nc.vector.tensor_add(
                    O_col[:, j + 1 : j + W],
                    O_col[:, j + 1 : j + W],
                    e_tiles[j][:, 0 : W - 1],
                )

            for i in range(NI):
                w0, w1 = i + 1, i + 1 + W
                relus = []
                for t in range(NT):
                    ab = relu_pool.tile([128, W], _F32R, tag="relu")
                    col = mt[t][:, i : i + 1].bitcast(_FP32)
                    if t < N_DVE:
                        nc.vector.tensor_scalar(
                            ab, mt[t][:, w0:w1], col, 0.0,
                            op0=AO.subtract, op1=AO.max,
                        )
                    else:
                        nc.scalar.activation(
                            ab, mt[t][:, w0:w1], AF.Relu, bias=col, scale=-1.0
                        )
                    relus.append(ab)
                l1 = psum_l1.tile([NK, W], _FP32, tag="pl1")
                for t in range(NT):
                    nc.tensor.matmul(
                        l1,
                        lhsT=S2_r[:, t * NK : (t + 1) * NK],
                        rhs=relus[t],
                        start=(t == 0),
                        stop=False,
                    )
                nc.tensor.matmul(
                    l1, lhsT=I64_r, rhs=Gsneg[:, w0:w1], start=False, stop=True
                )
                E = e_pool.tile([NK, W], _FP32, tag="ep")
                nc.scalar.activation(
                    E, l1, AF.Exp,
                    bias=Gsneg[:, i : i + 1].bitcast(_FP32),
                    scale=-1.0,
                    accum_out=O_row[:, i : i + 1],
                )
                e_tiles[i] = E
                if i >= LAG:
                    emit_col_add(i - LAG)
            for j in range(NI - LAG, NI):
                emit_col_add(j)

            nc.sync.dma_start(out=orow_d, in_=O_row)
            nc.sync.dma_start(out=ocol_d, in_=O_col)
    return nc


_CACHED = {}


def _get_program(repeat: int = 1):
    key = f"nc{repeat}"
    if key not in _CACHED:
        _CACHED[key] = build_program(repeat)
        _CACHED["S"] = build_s_matrices()
    return _CACHED[key], _CACHED["S"]


def make_in_maps(x: np.ndarray, T: np.ndarray, S2, Sg, I64):
    xT = np.ascontiguousarray(x.T.astype(np.float32, copy=False))
    T_arr = np.ascontiguousarray(T.astype(np.float32, copy=False))
    in_maps = []
    for c in range(N_CORES):
        xTc = np.ascontiguousarray(np.roll(xT, -NI * c, axis=1)[:, :JL])
        in_maps.append({"xT": xTc, "T": T_arr, "S2": S2, "Sg": Sg, "I64": I64})
    return in_maps


def assemble(results) -> np.ndarray:
    out = np.zeros((B, NK), dtype=np.float64)
    for c in range(N_CORES):
        R = results[c]["orow"]  # [k, i_local]
        C = results[c]["ocol"]  # [k, j_local]
        out[NI * c : NI * (c + 1), :] += R.T
        Cfull = np.zeros((B, NK), dtype=np.float64)
        Cfull[:JL] = C.T
        out += np.roll(Cfull, NI * c, axis=0)
    return out.astype(np.float32)


def run(x: np.ndarray, T: np.ndarray, trace: bool = False, repeat: int = 1):
    nc, (S2, Sg, I64) = _get_program(repeat)
    in_maps = make_in_maps(x, T, S2, Sg, I64)
    res = bass_utils.run_bass_kernel_spmd(
        nc, in_maps, core_ids=list(range(N_CORES)), trace=trace
    )
    return assemble(res.results), res


def kernel(x: np.ndarray, T: np.ndarray) -> np.ndarray:
    out, _ = run(x, T)
    return out



# revision 17
# speedup vs baseline: 1.8355x; 1.8355x over previous
"""MinibatchDiscrimination TRN2 kernel (v4).

x: [512, 1024] f32, T: [1024, 1024] f32.
M = (x @ T).reshape(512, 64, 16); l1[i,j,k] = sum_d |M[i,k,d]-M[j,k,d]|
out[i,k] = sum_j exp(-l1[i,j,k]) - 1.

Batch rows split across 8 cores (64 each), no collectives; each core's x^T
is rolled so its 64 rows sit at local columns 0..63 (JL=320 j-extent).
x^T and the column-permuted T are converted to bf16 host-side (halves DMA,
phase-1 matmuls run bf16 with no on-chip casts).

Pair coverage (per core, local rows i in [0,64), global circular):
  window w0 = i & ~1 (even-aligned), width 256.
  even i: d in [0,255]; odd i: d in [-1,254].
  Row accum: host-side sum over each DMA'd E2 tile (includes diagonal ->
  -1 on host; odd-row d=-1 dup covers pair {i-1,i} row-side).
  Col accum: E[:, 2:256] for every row (adds to out[w0+2 .. w0+255]).
  Missing pairs via an extras pass, each end row-side only:
    X1: partner i+256 (d=256); X2: partner i+257/i+255 (d=255, odd low end).

kd-permutation: tile t holds (k, d) for d in {2t, 2t+1}, partition
p = 2k + (d-2t), so the d-sum weight matrix S[p,k] = +-2*(k == p//2) is
identical for every tile (T's columns are permuted host-side).

|z| via relu with per-tile sign s_t (SIGNS): l1 = sum_t 2*sum_d
relu(s_t z) - Gs_j + Gs_i, Gs = sum_t s_t G_t.  Engine variants per tile:
  DVE s=+1: max(M_j - M_i, 0)            -> weight +2 (S2P)
  DVE s=-1: min(M_j - M_i, 0)            -> weight -2 (S2N)
  ACT s=-1: Relu(-(M_j) + M_i)           -> weight +2 (S2P)
-Gs_j is injected into the PSUM chain by a (-I) matmul; +Gs_i rides the
exp per-partition bias.  Values that must cancel exactly on the diagonal
round through bf16.  M tiles t5/t6 stay resident in PSUM so ACT's relu
reads use the cheaper PSUM access path.

Phase 2 packs row pairs (2r, 2r+1) into one PSUM tile (partitions 0:64 /
64:128): the S-matmuls col-tile into concurrent halves, one Exp covers
both rows, E[:,2:256] accumulates into a PSUM O_col via identity-matmul
injects (per-half tile positions (0,0)/(64,64) keep each accumulation
group at a single position - mixing positions in one group hangs the HW).
"""

import os

import numpy as np
import ml_dtypes

import concourse.bass as bass
import concourse.tile as tile
from concourse import mybir
from concourse import bass_utils

B = 512
F = 1024
KD = 1024
NK = 64
DK = 16
N_CORES = 8
NI = B // N_CORES  # 64 local rows
NT = KD // 128  # 8 kd tiles
NF = F // 128  # 8 f chunks
W = 256
JL = NI + W  # 320
NPAIR = NI // 2  # 32

_FP32 = mybir.dt.float32
_BF16 = mybir.dt.bfloat16

AF = mybir.ActivationFunctionType
AO = mybir.AluOpType

# per-tile signs: s_t = -1 for tiles that may run ACT's Relu(-z) variant
SIGNS = [1.0, 1.0, 1.0, 1.0, -1.0, -1.0, -1.0, 1.0]
# ACT tiles: t5/t6 (PSUM-resident M) for both halves, t4 for half 0 only
PSUM_MT = (5, 6)


def engine_for(half, t):
    if t in PSUM_MT:
        return "A"
    if t == 4 and half == 0:
        return "A"
    return "V"


def _split_all_waits(nc):
    """walrus in this env encodes at most 1 sync wait per instruction: hoist
    extra waits onto same-engine NOPs inserted just before the instruction."""
    count = 0
    for fn in nc.m.functions:
        for bb in fn.blocks:
            insts = list(bb.instructions)
            new = []
            changed = False
            for inst in insts:
                si = getattr(inst, "sync_info", None)
                waits = list(si.on_wait) if (si is not None and si.on_wait) else []
                if len(waits) > 1:
                    for w in waits[:-1]:
                        nop = mybir.InstNoOp(name=f"NOPW-{count}", ins=[], outs=[])
                        count += 1
                        nop.engine = inst.engine
                        nop.sync_info = mybir.SyncInfo(on_wait=[w], on_update=[])
                        nc.register_instruction(nop, overwrite=True)
                        new.append(nop)
                    si.on_wait = [waits[-1]]
                    changed = True
                new.append(inst)
            if changed:
                bb.instructions[:] = new


def _patch_drain_wait_limit():
    if getattr(tile.TileContext, "_wait_split_patched", False):
        return
    orig = tile.TileContext.schedule_and_allocate

    def schedule_and_allocate(self, *a, **k):
        r = orig(self, *a, **k)
        _split_all_waits(self.nc)
        return r

    tile.TileContext.schedule_and_allocate = schedule_and_allocate
    tile.TileContext._wait_split_patched = True


def build_host_consts():
    S2P = np.zeros((128, NK), dtype=np.float32)
    for p in range(128):
        S2P[p, p // 2] = 2.0
    S2N = -S2P
    Sg = np.zeros((128, NT * NK), dtype=np.float32)
    for p in range(128):
        for t in range(NT):
            Sg[p, t * NK + p // 2] = SIGNS[t]
    S1 = S2P / 2.0
    I64 = np.concatenate([np.eye(NK, dtype=np.float32)] * 2, axis=0)  # [128,64]
    NI64 = -np.eye(NK, dtype=np.float32)  # [64,64]
    perm = np.empty(KD, dtype=np.int64)
    for t in range(NT):
        for p in range(128):
            perm[t * 128 + p] = (p // 2) * DK + 2 * t + (p % 2)
    consts = {"S2P": S2P, "S2N": S2N, "S1": S1, "Sg": Sg, "I64": I64,
              "NI64": NI64}
    return consts, perm


def build_program():
    _patch_drain_wait_limit()
    nc = bass.Bass(
        "TRN2", target_bir_lowering=False, debug=False, num_devices=N_CORES
    )
    xT_d = nc.dram_tensor("xT", [F, JL], _BF16, kind="ExternalInput").ap()
    T_d = nc.dram_tensor("Tp", [F, KD], _BF16, kind="ExternalInput").ap()
    cdefs = [("S2P", [128, NK]), ("S2N", [128, NK]), ("S1", [128, NK]),
             ("Sg", [128, NT * NK]), ("I64", [128, NK]), ("NI64", [NK, NK])]
    c_d = {n: nc.dram_tensor(n, s, _FP32, kind="ExternalInput").ap()
           for n, s in cdefs}
    ocol_d = nc.dram_tensor("ocol", [NK, JL], _FP32, kind="ExternalOutput").ap()
    e2_d = nc.dram_tensor("e2", [128, NPAIR * W], _BF16, kind="ExternalOutput").ap()
    e12_d = nc.dram_tensor("e12", [128, NI], _FP32, kind="ExternalOutput").ap()

    with tile.TileContext(nc) as tc:
        with (
            tc.tile_pool(name="tstage", bufs=2) as tstage,
            tc.tile_pool(name="tr", bufs=NF) as tr_pool,
            tc.tile_pool(name="xr", bufs=NF) as xr_pool,
            tc.tile_pool(name="mt", bufs=1) as mt_pool,
            tc.tile_pool(name="consts", bufs=1) as c_pool,
            tc.tile_pool(name="abs", bufs=8) as abs_pool,
            tc.tile_pool(name="e2", bufs=8) as e2_pool,
            tc.tile_pool(name="outs", bufs=1) as o_pool,
            tc.tile_pool(name="pmm", bufs=2, space="PSUM") as psum_mm,
            tc.tile_pool(name="pmt", bufs=2, space="PSUM") as psum_mt,
            tc.tile_pool(name="ppair", bufs=3, space="PSUM") as psum_pair,
            tc.tile_pool(name="pocol", bufs=1, space="PSUM") as psum_ocol,
        ):
            # ---- loads (bf16, no casts) ----
            T_r = []
            for f in range(NF):
                tr = tr_pool.tile([128, KD], _BF16, tag="tr")
                nc.sync.dma_start(
                    out=tr[:, 0:512], in_=T_d[f * 128 : (f + 1) * 128, 0:512]
                )
                nc.sync.dma_start(
                    out=tr[:, 512:KD], in_=T_d[f * 128 : (f + 1) * 128, 512:KD]
                )
                T_r.append(tr)
            x_r = []
            for f in range(NF):
                xr = xr_pool.tile([128, JL], _BF16, tag="xr")
                nc.sync.dma_start(out=xr, in_=xT_d[f * 128 : (f + 1) * 128, :])
                x_r.append(xr)

            cb = {}
            for name, shape in cdefs:
                st = tstage.tile(shape, _FP32, tag="cstage")
                nc.sync.dma_start(out=st, in_=c_d[name])
                tl = c_pool.tile(shape, _BF16, tag=name.lower())
                nc.vector.tensor_copy(tl, st)
                cb[name] = tl
            S2Pb, S2Nb, S1b = cb["S2P"], cb["S2N"], cb["S1"]
            Sgb, I64b, NI64b = cb["Sg"], cb["I64"], cb["NI64"]

            # ---- phase 1: M^T tiles -> mt_all (bf16) + mcols (fp32);
            # tiles t5/t6 stay in PSUM for ACT's relu reads ----
            mt_all = mt_pool.tile([128, NT * JL], _BF16, tag="mt")
            mcols = mt_pool.tile([128, NT * NK], _FP32, tag="mcols")
            mt_ps = {}
            for t in range(NT):
                pool = psum_mt if t in PSUM_MT else psum_mm
                pm = pool.tile([128, JL], _FP32, tag="pm" if t not in PSUM_MT else "pmt")
                for f in range(NF):
                    nc.tensor.matmul(
                        pm,
                        lhsT=T_r[f][:, t * 128 : (t + 1) * 128],
                        rhs=x_r[f],
                        start=(f == 0),
                        stop=(f == NF - 1),
                    )
                if t in PSUM_MT:
                    mt_ps[t] = pm
                nc.scalar.activation(mt_all[:, t * JL : (t + 1) * JL], pm, AF.Copy)
                if t in PSUM_MT:
                    # ACT reads PSUM fp32 directly; bias must match exactly
                    nc.vector.tensor_copy(
                        mcols[:, t * NK : (t + 1) * NK], pm[:, 0:NK]
                    )
                else:
                    # bias rounds through bf16 so the diagonal is exactly 0
                    nc.vector.tensor_copy(
                        mcols[:, t * NK : (t + 1) * NK],
                        mt_all[:, t * JL : t * JL + NK],
                    )

            # ---- phase 1.5: signed G (Gs) and unsigned G (Gall) ----
            mtv = mt_all.rearrange("p (t w) -> p t w", t=NT)
            pgs = psum_mm.tile([NK, JL], _FP32, tag="pm")
            for t in range(NT):
                nc.tensor.matmul(
                    pgs,
                    lhsT=Sgb[:, t * NK : (t + 1) * NK],
                    rhs=mtv[:, t, :],
                    start=(t == 0),
                    stop=(t == NT - 1),
                )
            Gs_sb = c_pool.tile([NK, JL], _BF16, tag="gs")
            nc.scalar.activation(Gs_sb, pgs, AF.Copy)
            pga = psum_mm.tile([NK, JL], _FP32, tag="pm")
            for t in range(NT):
                nc.tensor.matmul(
                    pga,
                    lhsT=S1b,
                    rhs=mtv[:, t, :],
                    start=(t == 0),
                    stop=(t == NT - 1),
                )
            Gall_sb = c_pool.tile([NK, JL], _BF16, tag="gall")
            nc.scalar.activation(Gall_sb, pga, AF.Copy)
            # exp bias for pair r: GBneg[0:64, r] = -Gs[k, 2r],
            # GBneg[64:128, r] = -Gs[k, 2r+1]  (from the bf16-rounded Gs)
            GBneg = c_pool.tile([128, NPAIR], _FP32, tag="gbneg")
            nc.vector.tensor_scalar(
                GBneg[0:NK, :], Gs_sb[:, 0 : NI : 2], -1.0, None, op0=AO.mult
            )
            nc.vector.tensor_scalar(
                GBneg[NK:128, :], Gs_sb[:, 1 : NI : 2], -1.0, None, op0=AO.mult
            )
            Gp2 = c_pool.tile([NK, NK], _BF16, tag="gp2")
            nc.vector.tensor_copy(Gp2[:, 0 : NK : 2], Gall_sb[:, W + 1 : W + NK : 2])
            nc.vector.tensor_copy(Gp2[:, 1 : NK : 2], Gall_sb[:, W : W + NK - 1 : 2])

            # ---- phase 2: 32 row pairs ----
            ocol = psum_ocol.tile([128, JL], _FP32, tag="pocol")
            Zb = c_pool.tile([128, JL], _BF16, tag="zb")
            nc.vector.memset(Zb, 0.0)
            for half in range(2):
                nc.tensor.matmul(
                    ocol[half * NK : (half + 1) * NK, :],
                    lhsT=I64b[half * NK : (half + 1) * NK, :],
                    rhs=Zb[half * NK : (half + 1) * NK, :],
                    start=True,
                    stop=False,
                    skip_group_check=True,
                )

            def relu_tile_op(half, t, out_ap, i, w0):
                col = mcols[:, t * NK + i : t * NK + i + 1]
                if engine_for(half, t) == "A":
                    in_ap = (
                        mt_ps[t][:, w0 : w0 + W]
                        if t in PSUM_MT
                        else mt_all[:, t * JL + w0 : t * JL + w0 + W]
                    )
                    nc.scalar.activation(out_ap, in_ap, AF.Relu, bias=col, scale=-1.0)
                    return S2Pb
                in_ap = mt_all[:, t * JL + w0 : t * JL + w0 + W]
                if SIGNS[t] > 0:
                    nc.vector.tensor_scalar(
                        out_ap, in_ap, col, 0.0, op0=AO.subtract, op1=AO.max
                    )
                    return S2Pb
                nc.vector.tensor_scalar(
                    out_ap, in_ap, col, 0.0, op0=AO.subtract, op1=AO.min
                )
                return S2Nb

            for r in range(NPAIR):
                w0 = 2 * r
                ab0 = abs_pool.tile([128, NT * W], _BF16, tag="abs")
                ab1 = abs_pool.tile([128, NT * W], _BF16, tag="abs")
                ab = (ab0, ab1)
                wts = [[None] * NT, [None] * NT]
                for half in range(2):
                    i = 2 * r + half
                    for t in range(NT):
                        wts[half][t] = relu_tile_op(
                            half, t, ab[half][:, t * W : (t + 1) * W], i, w0
                        )
                pp = psum_pair.tile([128, W], _FP32, tag="ppair")
                for t in range(NT):
                    for half in range(2):
                        nc.tensor.matmul(
                            pp[half * NK : (half + 1) * NK, :],
                            lhsT=wts[half][t],
                            rhs=ab[half][:, t * W : (t + 1) * W],
                            start=(t == 0),
                            stop=False,
                            skip_group_check=True,
                        )
                for half in range(2):
                    nc.tensor.matmul(
                        pp[half * NK : (half + 1) * NK, :],
                        lhsT=NI64b,
                        rhs=Gs_sb[:, w0 : w0 + W],
                        start=False,
                        stop=True,
                        skip_group_check=True,
                    )
                E2 = e2_pool.tile([128, W], _BF16, tag="e2")
                nc.scalar.activation(
                    E2, pp, AF.Exp, scale=-1.0, bias=GBneg[:, r : r + 1]
                )
                nc.sync.dma_start(out=e2_d[:, r * W : (r + 1) * W], in_=E2)
                for half in range(2):
                    nc.tensor.matmul(
                        ocol[half * NK : (half + 1) * NK, w0 + 2 : w0 + W],
                        lhsT=I64b[half * NK : (half + 1) * NK, :],
                        rhs=E2[half * NK : (half + 1) * NK, 2:W],
                        start=False,
                        stop=(r == NPAIR - 1),
                        skip_group_check=True,
                    )

            # ---- extras: d=256 (X1) and odd-d=255 (X2) pairs ----
            X1 = abs_pool.tile([128, NT * NK], _BF16, tag="x1")
            X2 = abs_pool.tile([128, NT * NK], _BF16, tag="x2")
            x1v = X1.rearrange("p (t w) -> p t w", t=NT)
            x2v = X2.rearrange("p (t w) -> p t w", t=NT)
            nc.vector.tensor_tensor(
                x1v, mtv[:, :, W : W + NK], mtv[:, :, 0:NK], op=AO.subtract
            )
            nc.vector.tensor_tensor(
                x2v[:, :, 0:NK:2],
                mtv[:, :, W + 1 : W + NK : 2],
                mtv[:, :, 0:NK:2],
                op=AO.subtract,
            )
            nc.vector.tensor_tensor(
                x2v[:, :, 1:NK:2],
                mtv[:, :, W : W + NK - 1 : 2],
                mtv[:, :, 1:NK:2],
                op=AO.subtract,
            )
            nc.vector.tensor_scalar(X1, X1, 0.0, None, op0=AO.max)
            nc.vector.tensor_scalar(X2, X2, 0.0, None, op0=AO.max)
            px = psum_pair.tile([128, NK], _FP32, tag="ppair")
            for t in range(NT):
                for half, X in enumerate((X1, X2)):
                    nc.tensor.matmul(
                        px[half * NK : (half + 1) * NK, :],
                        lhsT=S2Pb,
                        rhs=X[:, t * NK : (t + 1) * NK],
                        start=(t == 0),
                        stop=False,
                        skip_group_check=True,
                    )
            # l1x = S2-sum - Gall_partner + Gall_i
            for half, gpart in enumerate((Gall_sb[:, W : W + NK], Gp2[:, :])):
                nc.tensor.matmul(
                    px[half * NK : (half + 1) * NK, :],
                    lhsT=NI64b,
                    rhs=gpart,
                    start=False,
                    stop=False,
                    skip_group_check=True,
                )
            for half in range(2):
                nc.tensor.matmul(
                    px[half * NK : (half + 1) * NK, :],
                    lhsT=I64b[0:NK, :],
                    rhs=Gall_sb[:, 0:NK],
                    start=False,
                    stop=True,
                    skip_group_check=True,
                )
            E12 = o_pool.tile([128, NI], _FP32, tag="e12")
            nc.scalar.activation(E12, px, AF.Exp, scale=-1.0)

            ocol_sb = o_pool.tile([NK, JL], _FP32, tag="ocolsb")
            nc.scalar.activation(ocol_sb, ocol[0:NK, :], AF.Copy)
            nc.vector.tensor_tensor(
                ocol_sb, ocol_sb, ocol[NK:128, :], op=AO.add
            )
            nc.sync.dma_start(out=e12_d, in_=E12)
            nc.sync.dma_start(out=ocol_d, in_=ocol_sb)
    return nc


_CACHED = {}


def _get_program():
    if "nc" not in _CACHED:
        _CACHED["nc"] = build_program()
        _CACHED["consts"] = build_host_consts()
    return _CACHED["nc"], _CACHED["consts"]


def make_in_maps(x: np.ndarray, T: np.ndarray, consts, perm):
    bf16 = ml_dtypes.bfloat16
    xT = np.ascontiguousarray(x.T.astype(np.float32, copy=False))
    Tp = np.ascontiguousarray(
        T.astype(np.float32, copy=False)[:, perm].astype(bf16)
    )
    in_maps = []
    for c in range(N_CORES):
        xTc = np.ascontiguousarray(
            np.roll(xT, -NI * c, axis=1)[:, :JL].astype(bf16)
        )
        m = {"xT": xTc, "Tp": Tp}
        m.update(consts)
        in_maps.append(m)
    return in_maps


def assemble(results) -> np.ndarray:
    out = np.zeros((B, NK), dtype=np.float64)
    for c in range(N_CORES):
        E2 = np.asarray(results[c]["e2"]).astype(np.float32)  # [128, NPAIR*W]
        E12 = results[c]["e12"]  # [128, NI]
        C = results[c]["ocol"]  # [NK, JL]
        base = NI * c
        R = E2.reshape(128, NPAIR, W).sum(axis=2)  # [128, NPAIR]
        for half in range(2):
            rows = base + 2 * np.arange(NPAIR) + half
            out[rows, :] += R[half * NK : (half + 1) * NK, :].T
        out[base : base + NI, :] += E12[0:NK, :].T
        out[base : base + NI, :] += E12[NK:128, :].T
        Cfull = np.zeros((B, NK), dtype=np.float64)
        Cfull[:JL] = C.T
        out += np.roll(Cfull, base, axis=0)
    out -= 1.0  # diagonal exp(0) included in row sums
    return out.astype(np.float32)


def run(x: np.ndarray, T: np.ndarray, trace: bool = False):
    nc, (consts, perm) = _get_program()
    in_maps = make_in_maps(x, T, consts, perm)
    res = bass_utils.run_bass_kernel_spmd(
        nc, in_maps, core_ids=list(range(N_CORES)), trace=trace
    )
    return assemble(res.results), res


def kernel(x: np.ndarray, T: np.ndarray) -> np.ndarray:
    out, _ = run(x, T)
    return out


# revision 21
# speedup vs baseline: 1.8887x; 1.0290x over previous
"""MinibatchDiscrimination TRN2 kernel (v4).

x: [512, 1024] f32, T: [1024, 1024] f32.
M = (x @ T).reshape(512, 64, 16); l1[i,j,k] = sum_d |M[i,k,d]-M[j,k,d]|
out[i,k] = sum_j exp(-l1[i,j,k]) - 1.

Batch rows split across 8 cores (64 each), no collectives; each core's x^T
is rolled so its 64 rows sit at local columns 0..63 (JL=320 j-extent).
x^T and the column-permuted T are converted to bf16 host-side (halves DMA,
phase-1 matmuls run bf16 with no on-chip casts).

Pair coverage (per core, local rows i in [0,64), global circular):
  window w0 = i & ~1 (even-aligned), width 256.
  even i: d in [0,255]; odd i: d in [-1,254].
  Row accum: host-side sum over each DMA'd E2 tile (includes diagonal ->
  -1 on host; odd-row d=-1 dup covers pair {i-1,i} row-side).
  Col accum: E[:, 2:256] for every row (adds to out[w0+2 .. w0+255]).
  Missing pairs via an extras pass, each end row-side only:
    X1: partner i+256 (d=256); X2: partner i+257/i+255 (d=255, odd low end).

kd-permutation: tile t holds (k, d) for d in {2t, 2t+1}, partition
p = 2k + (d-2t), so the d-sum weight matrix S[p,k] = +-2*(k == p//2) is
identical for every tile (T's columns are permuted host-side).

|z| via relu with per-tile sign s_t (SIGNS): l1 = sum_t 2*sum_d
relu(s_t z) - Gs_j + Gs_i, Gs = sum_t s_t G_t.  Engine variants per tile:
  DVE s=+1: max(M_j - M_i, 0)            -> weight +2 (S2P)
  DVE s=-1: min(M_j - M_i, 0)            -> weight -2 (S2N)
  ACT s=-1: Relu(-(M_j) + M_i)           -> weight +2 (S2P)
-Gs_j is injected into the PSUM chain by a (-I) matmul; +Gs_i rides the
exp per-partition bias.  Values that must cancel exactly on the diagonal
round through bf16.  M tiles t5/t6 stay resident in PSUM so ACT's relu
reads use the cheaper PSUM access path.

Phase 2 packs row pairs (2r, 2r+1) into one PSUM tile (partitions 0:64 /
64:128): the S-matmuls col-tile into concurrent halves, one Exp covers
both rows, E[:,2:256] accumulates into a PSUM O_col via identity-matmul
injects (per-half tile positions (0,0)/(64,64) keep each accumulation
group at a single position - mixing positions in one group hangs the HW).
"""

import os

import numpy as np
import ml_dtypes

import concourse.bass as bass
import concourse.tile as tile
from concourse import mybir
from concourse import bass_utils

B = 512
F = 1024
KD = 1024
NK = 64
DK = 16
N_CORES = 8
NI = B // N_CORES  # 64 local rows
NT = KD // 128  # 8 kd tiles
NF = F // 128  # 8 f chunks
W = 256
JL = NI + W  # 320
NPAIR = NI // 2  # 32

_FP32 = mybir.dt.float32
_BF16 = mybir.dt.bfloat16

AF = mybir.ActivationFunctionType
AO = mybir.AluOpType

# per-tile signs: s_t = -1 for tiles that may run ACT's Relu(-z) variant
SIGNS = [1.0, 1.0, 1.0, 1.0, -1.0, -1.0, -1.0, 1.0]
# ACT tiles: t5/t6 (PSUM-resident M) for both halves, t4 for half 0 only
PSUM_MT = (5, 6)


def engine_for(half, t):
    if t in PSUM_MT:
        return "A"
    if t == 4 and half == 0:
        return "A"
    return "V"


def _split_all_waits(nc):
    """walrus in this env encodes at most 1 sync wait per instruction: hoist
    extra waits onto same-engine NOPs inserted just before the instruction."""
    count = 0
    for fn in nc.m.functions:
        for bb in fn.blocks:
            insts = list(bb.instructions)
            new = []
            changed = False
            for inst in insts:
                si = getattr(inst, "sync_info", None)
                waits = list(si.on_wait) if (si is not None and si.on_wait) else []
                if len(waits) > 1:
                    for w in waits[:-1]:
                        nop = mybir.InstNoOp(name=f"NOPW-{count}", ins=[], outs=[])
                        count += 1
                        nop.engine = inst.engine
                        nop.sync_info = mybir.SyncInfo(on_wait=[w], on_update=[])
                        nc.register_instruction(nop, overwrite=True)
                        new.append(nop)
                    si.on_wait = [waits[-1]]
                    changed = True
                new.append(inst)
            if changed:
                bb.instructions[:] = new


def _patch_drain_wait_limit():
    if getattr(tile.TileContext, "_wait_split_patched", False):
        return
    orig = tile.TileContext.schedule_and_allocate

    def schedule_and_allocate(self, *a, **k):
        r = orig(self, *a, **k)
        _split_all_waits(self.nc)
        return r

    tile.TileContext.schedule_and_allocate = schedule_and_allocate
    tile.TileContext._wait_split_patched = True


def build_host_consts():
    S2P = np.zeros((128, NK), dtype=np.float32)
    for p in range(128):
        S2P[p, p // 2] = 2.0
    S2N = -S2P
    Sg = np.zeros((128, NT * NK), dtype=np.float32)
    for p in range(128):
        for t in range(NT):
            Sg[p, t * NK + p // 2] = SIGNS[t]
    S1 = S2P / 2.0
    I64 = np.concatenate([np.eye(NK, dtype=np.float32)] * 2, axis=0)  # [128,64]
    NI64 = -np.eye(NK, dtype=np.float32)  # [64,64]
    perm = np.empty(KD, dtype=np.int64)
    for t in range(NT):
        for p in range(128):
            perm[t * 128 + p] = (p // 2) * DK + 2 * t + (p % 2)
    consts = {"S2P": S2P, "S2N": S2N, "S1": S1, "Sg": Sg, "I64": I64,
              "NI64": NI64}
    return consts, perm


def build_program():
    _patch_drain_wait_limit()
    nc = bass.Bass(
        "TRN2", target_bir_lowering=False, debug=False, num_devices=N_CORES
    )
    xT_d = nc.dram_tensor("xT", [F, JL], _BF16, kind="ExternalInput").ap()
    T_d = nc.dram_tensor("Tp", [F, KD], _BF16, kind="ExternalInput").ap()
    cdefs = [("S2P", [128, NK]), ("S2N", [128, NK]), ("S1", [128, NK]),
             ("Sg", [128, NT * NK]), ("I64", [128, NK]), ("NI64", [NK, NK])]
    call_d = nc.dram_tensor("Call", [128, 832], _FP32, kind="ExternalInput").ap()
    ocol_d = nc.dram_tensor("ocol", [NK, JL], _FP32, kind="ExternalOutput").ap()
    e2_d = nc.dram_tensor("e2", [128, NPAIR * W], _BF16, kind="ExternalOutput").ap()
    e12_d = nc.dram_tensor("e12", [128, NI], _FP32, kind="ExternalOutput").ap()

    with tile.TileContext(nc) as tc:
        with (
            tc.tile_pool(name="tstage", bufs=2) as tstage,
            tc.tile_pool(name="tr", bufs=NF) as tr_pool,
            tc.tile_pool(name="xr", bufs=NF) as xr_pool,
            tc.tile_pool(name="mt", bufs=1) as mt_pool,
            tc.tile_pool(name="consts", bufs=1) as c_pool,
            tc.tile_pool(name="abs", bufs=8) as abs_pool,
            tc.tile_pool(name="outs", bufs=1) as o_pool,
            tc.tile_pool(name="pmm", bufs=2, space="PSUM") as psum_mm,
            tc.tile_pool(name="pmt", bufs=2, space="PSUM") as psum_mt,
            tc.tile_pool(name="ppair", bufs=3, space="PSUM") as psum_pair,
            tc.tile_pool(name="pocol", bufs=1, space="PSUM") as psum_ocol,
        ):
            # ---- loads (bf16, no casts); descriptor issue split across
            # sync+vector engines, one fat descriptor per T chunk ----
            T_r = []
            for f in range(NF):
                tr = tr_pool.tile([128, KD], _BF16, tag="tr")
                eng = nc.sync if f % 2 == 0 else nc.scalar
                eng.dma_start(out=tr, in_=T_d[f * 128 : (f + 1) * 128, :])
                T_r.append(tr)
            x_all = xr_pool.tile([128, NF * JL], _BF16, tag="xr")
            xv = x_all.rearrange("p (f c) -> p f c", f=NF)
            xs = xT_d.rearrange("(f p) c -> p f c", f=NF)
            nc.sync.dma_start(out=xv[:, 0:4, :], in_=xs[:, 0:4, :])
            nc.scalar.dma_start(out=xv[:, 4:NF, :], in_=xs[:, 4:NF, :])
            x_r = [xv[:, f, :] for f in range(NF)]

            cb = {}
            cst = tstage.tile([128, 832], _FP32, tag="cstage")
            nc.sync.dma_start(out=cst, in_=call_d)
            off = 0
            for name, shape in cdefs:
                tl = c_pool.tile(shape, _BF16, tag=name.lower())
                if shape[0] == 128:
                    nc.vector.tensor_copy(tl, cst[:, off : off + shape[1]])
                else:
                    nc.vector.tensor_copy(tl, cst[0 : shape[0], off : off + shape[1]])
                off += shape[1]
                cb[name] = tl
            S2Pb, S2Nb, S1b = cb["S2P"], cb["S2N"], cb["S1"]
            Sgb, I64b, NI64b = cb["Sg"], cb["I64"], cb["NI64"]

            # ---- phase 1: M^T tiles -> mt_all (bf16) + mcols (fp32);
            # tiles t5/t6 stay in PSUM for ACT's relu reads ----
            mt_all = mt_pool.tile([128, NT * JL], _BF16, tag="mt")
            mcols = mt_pool.tile([128, NT * NK], _FP32, tag="mcols")
            mt_ps = {}
            for t in range(NT):
                pool = psum_mt if t in PSUM_MT else psum_mm
                pm = pool.tile([128, JL], _FP32, tag="pm" if t not in PSUM_MT else "pmt")
                for f in range(NF):
                    nc.tensor.matmul(
                        pm,
                        lhsT=T_r[f][:, t * 128 : (t + 1) * 128],
                        rhs=x_r[f],
                        start=(f == 0),
                        stop=(f == NF - 1),
                    )
                if t in PSUM_MT:
                    mt_ps[t] = pm
                nc.scalar.activation(mt_all[:, t * JL : (t + 1) * JL], pm, AF.Copy)
                if t in PSUM_MT:
                    # ACT reads PSUM fp32 directly; bias must match exactly
                    nc.vector.tensor_copy(
                        mcols[:, t * NK : (t + 1) * NK], pm[:, 0:NK]
                    )
                else:
                    # bias rounds through bf16 so the diagonal is exactly 0
                    nc.vector.tensor_copy(
                        mcols[:, t * NK : (t + 1) * NK],
                        mt_all[:, t * JL : t * JL + NK],
                    )

            # ---- phase 1.5: signed G (Gs) and unsigned G (Gall) ----
            mtv = mt_all.rearrange("p (t w) -> p t w", t=NT)
            pgs = psum_mm.tile([NK, JL], _FP32, tag="pm")
            for t in range(NT):
                nc.tensor.matmul(
                    pgs,
                    lhsT=Sgb[:, t * NK : (t + 1) * NK],
                    rhs=mtv[:, t, :],
                    start=(t == 0),
                    stop=(t == NT - 1),
                )
            Gs_sb = c_pool.tile([NK, JL], _BF16, tag="gs")
            nc.scalar.activation(Gs_sb, pgs, AF.Copy)
            pga = psum_mm.tile([NK, JL], _FP32, tag="pm")
            for t in range(NT):
                nc.tensor.matmul(
                    pga,
                    lhsT=S1b,
                    rhs=mtv[:, t, :],
                    start=(t == 0),
                    stop=(t == NT - 1),
                )
            Gall_sb = c_pool.tile([NK, JL], _BF16, tag="gall")
            nc.scalar.activation(Gall_sb, pga, AF.Copy)
            # exp bias for pair r: GBneg[0:64, r] = -Gs[k, 2r],
            # GBneg[64:128, r] = -Gs[k, 2r+1]  (from the bf16-rounded Gs)
            GBneg = c_pool.tile([128, NPAIR], _FP32, tag="gbneg")
            nc.vector.tensor_scalar(
                GBneg[0:NK, :], Gs_sb[:, 0 : NI : 2], -1.0, None, op0=AO.mult
            )
            nc.vector.tensor_scalar(
                GBneg[NK:128, :], Gs_sb[:, 1 : NI : 2], -1.0, None, op0=AO.mult
            )
            Gp2 = c_pool.tile([NK, NK], _BF16, tag="gp2")
            nc.vector.tensor_copy(Gp2[:, 0 : NK : 2], Gall_sb[:, W + 1 : W + NK : 2])
            nc.vector.tensor_copy(Gp2[:, 1 : NK : 2], Gall_sb[:, W : W + NK - 1 : 2])

            # ---- phase 2: 32 row pairs ----
            ocol = psum_ocol.tile([128, JL], _FP32, tag="pocol")
            Zb = c_pool.tile([128, JL], _BF16, tag="zb")
            nc.vector.memset(Zb, 0.0)
            for half in range(2):
                nc.tensor.matmul(
                    ocol[half * NK : (half + 1) * NK, :],
                    lhsT=I64b[half * NK : (half + 1) * NK, :],
                    rhs=Zb[half * NK : (half + 1) * NK, :],
                    start=True,
                    stop=False,
                    skip_group_check=True,
                )

            def relu_tile_op(half, t, out_ap, i, w0):
                col = mcols[:, t * NK + i : t * NK + i + 1]
                if engine_for(half, t) == "A":
                    in_ap = (
                        mt_ps[t][:, w0 : w0 + W]
                        if t in PSUM_MT
                        else mt_all[:, t * JL + w0 : t * JL + w0 + W]
                    )
                    nc.scalar.activation(out_ap, in_ap, AF.Relu, bias=col, scale=-1.0)
                    return S2Pb
                in_ap = mt_all[:, t * JL + w0 : t * JL + w0 + W]
                if SIGNS[t] > 0:
                    nc.vector.tensor_scalar(
                        out_ap, in_ap, col, 0.0, op0=AO.subtract, op1=AO.max
                    )
                    return S2Pb
                nc.vector.tensor_scalar(
                    out_ap, in_ap, col, 0.0, op0=AO.subtract, op1=AO.min
                )
                return S2Nb

            # ---- extras: d=256 (X1) and odd-d=255 (X2) pairs ----
            X1 = abs_pool.tile([128, NT * NK], _BF16, tag="x1")
            X2 = abs_pool.tile([128, NT * NK], _BF16, tag="x2")
            x1v = X1.rearrange("p (t w) -> p t w", t=NT)
            x2v = X2.rearrange("p (t w) -> p t w", t=NT)
            nc.vector.tensor_tensor(
                x1v, mtv[:, :, W : W + NK], mtv[:, :, 0:NK], op=AO.subtract
            )
            nc.vector.tensor_tensor(
                x2v[:, :, 0:NK:2],
                mtv[:, :, W + 1 : W + NK : 2],
                mtv[:, :, 0:NK:2],
                op=AO.subtract,
            )
            nc.vector.tensor_tensor(
                x2v[:, :, 1:NK:2],
                mtv[:, :, W : W + NK - 1 : 2],
                mtv[:, :, 1:NK:2],
                op=AO.subtract,
            )
            nc.vector.tensor_scalar(X1, X1, 0.0, None, op0=AO.max)
            nc.vector.tensor_scalar(X2, X2, 0.0, None, op0=AO.max)
            px = psum_pair.tile([128, NK], _FP32, tag="ppair")
            for t in range(NT):
                for half, X in enumerate((X1, X2)):
                    nc.tensor.matmul(
                        px[half * NK : (half + 1) * NK, :],
                        lhsT=S2Pb,
                        rhs=X[:, t * NK : (t + 1) * NK],
                        start=(t == 0),
                        stop=False,
                        skip_group_check=True,
                    )
            # l1x = S2-sum - Gall_partner + Gall_i
            for half, gpart in enumerate((Gall_sb[:, W : W + NK], Gp2[:, :])):
                nc.tensor.matmul(
                    px[half * NK : (half + 1) * NK, :],
                    lhsT=NI64b,
                    rhs=gpart,
                    start=False,
                    stop=False,
                    skip_group_check=True,
                )
            for half in range(2):
                nc.tensor.matmul(
                    px[half * NK : (half + 1) * NK, :],
                    lhsT=I64b[0:NK, :],
                    rhs=Gall_sb[:, 0:NK],
                    start=False,
                    stop=True,
                    skip_group_check=True,
                )
            E12 = o_pool.tile([128, NI], _FP32, tag="e12")
            nc.scalar.activation(E12, px, AF.Exp, scale=-1.0)


            E2big = mt_pool.tile([128, NPAIR * W], _BF16, tag="e2big")
            for r in range(NPAIR):
                w0 = 2 * r
                ab0 = abs_pool.tile([128, NT * W], _BF16, tag="abs")
                ab1 = abs_pool.tile([128, NT * W], _BF16, tag="abs")
                ab = (ab0, ab1)
                wts = [[None] * NT, [None] * NT]
                for half in range(2):
                    i = 2 * r + half
                    for t in range(NT):
                        wts[half][t] = relu_tile_op(
                            half, t, ab[half][:, t * W : (t + 1) * W], i, w0
                        )
                pp = psum_pair.tile([128, W], _FP32, tag="ppair")
                for t in range(NT):
                    for half in range(2):
                        nc.tensor.matmul(
                            pp[half * NK : (half + 1) * NK, :],
                            lhsT=wts[half][t],
                            rhs=ab[half][:, t * W : (t + 1) * W],
                            start=(t == 0),
                            stop=False,
                            skip_group_check=True,
                        )
                for half in range(2):
                    nc.tensor.matmul(
                        pp[half * NK : (half + 1) * NK, :],
                        lhsT=NI64b,
                        rhs=Gs_sb[:, w0 : w0 + W],
                        start=False,
                        stop=True,
                        skip_group_check=True,
                    )
                E2 = E2big[:, r * W : (r + 1) * W]
                nc.scalar.activation(
                    E2, pp, AF.Exp, scale=-1.0, bias=GBneg[:, r : r + 1]
                )
                for half in range(2):
                    nc.tensor.matmul(
                        ocol[half * NK : (half + 1) * NK, w0 + 2 : w0 + W],
                        lhsT=I64b[half * NK : (half + 1) * NK, :],
                        rhs=E2[half * NK : (half + 1) * NK, 2:W],
                        start=False,
                        stop=(r == NPAIR - 1),
                        skip_group_check=True,
                    )
                if r % 8 == 7:
                    nc.sync.dma_start(
                        out=e2_d[:, (r - 7) * W : (r + 1) * W],
                        in_=E2big[:, (r - 7) * W : (r + 1) * W],
                    )

            ocol_sb = o_pool.tile([NK, JL], _FP32, tag="ocolsb")
            nc.scalar.activation(ocol_sb, ocol[0:NK, :], AF.Copy)
            nc.vector.tensor_tensor(
                ocol_sb, ocol_sb, ocol[NK:128, :], op=AO.add
            )
            nc.sync.dma_start(out=e12_d, in_=E12)
            nc.sync.dma_start(out=ocol_d, in_=ocol_sb)
    return nc


_CACHED = {}


def _get_program():
    if "nc" not in _CACHED:
        _CACHED["nc"] = build_program()
        _CACHED["consts"] = build_host_consts()
    return _CACHED["nc"], _CACHED["consts"]


def make_in_maps(x: np.ndarray, T: np.ndarray, consts, perm):
    bf16 = ml_dtypes.bfloat16
    xT = np.ascontiguousarray(x.T.astype(np.float32, copy=False))
    Tp = np.ascontiguousarray(
        T.astype(np.float32, copy=False)[:, perm].astype(bf16)
    )
    Call = np.zeros((128, 832), dtype=np.float32)
    off = 0
    for name in ("S2P", "S2N", "S1", "Sg", "I64", "NI64"):
        arr = consts[name]
        Call[0 : arr.shape[0], off : off + arr.shape[1]] = arr
        off += arr.shape[1]
    in_maps = []
    for c in range(N_CORES):
        xTc = np.ascontiguousarray(
            np.roll(xT, -NI * c, axis=1)[:, :JL].astype(bf16)
        )
        in_maps.append({"xT": xTc, "Tp": Tp, "Call": Call})
    return in_maps


def assemble(results) -> np.ndarray:
    out = np.zeros((B, NK), dtype=np.float64)
    for c in range(N_CORES):
        E2 = np.asarray(results[c]["e2"]).astype(np.float32)  # [128, NPAIR*W]
        E12 = results[c]["e12"]  # [128, NI]
        C = results[c]["ocol"]  # [NK, JL]
        base = NI * c
        R = E2.reshape(128, NPAIR, W).sum(axis=2)  # [128, NPAIR]
        for half in range(2):
            rows = base + 2 * np.arange(NPAIR) + half
            out[rows, :] += R[half * NK : (half + 1) * NK, :].T
        out[base : base + NI, :] += E12[0:NK, :].T
        out[base : base + NI, :] += E12[NK:128, :].T
        Cfull = np.zeros((B, NK), dtype=np.float64)
        Cfull[:JL] = C.T
        out += np.roll(Cfull, base, axis=0)
    out -= 1.0  # diagonal exp(0) included in row sums
    return out.astype(np.float32)


def run(x: np.ndarray, T: np.ndarray, trace: bool = False):
    nc, (consts, perm) = _get_program()
    in_maps = make_in_maps(x, T, consts, perm)
    res = bass_utils.run_bass_kernel_spmd(
        nc, in_maps, core_ids=list(range(N_CORES)), trace=trace
    )
    return assemble(res.results), res


def kernel(x: np.ndarray, T: np.ndarray) -> np.ndarray:
    out, _ = run(x, T)
    return out


# revision 22
# speedup vs baseline: 1.9536x; 1.0344x over previous
"""MinibatchDiscrimination TRN2 kernel (v4).

x: [512, 1024] f32, T: [1024, 1024] f32.
M = (x @ T).reshape(512, 64, 16); l1[i,j,k] = sum_d |M[i,k,d]-M[j,k,d]|
out[i,k] = sum_j exp(-l1[i,j,k]) - 1.

Batch rows split across 8 cores (64 each), no collectives; each core's x^T
is rolled so its 64 rows sit at local columns 0..63 (JL=320 j-extent).
x^T and the column-permuted T are converted to bf16 host-side (halves DMA,
phase-1 matmuls run bf16 with no on-chip casts).

Pair coverage (per core, local rows i in [0,64), global circular):
  window w0 = i & ~1 (even-aligned), width 256.
  even i: d in [0,255]; odd i: d in [-1,254].
  Row accum: host-side sum over each DMA'd E2 tile (includes diagonal ->
  -1 on host; odd-row d=-1 dup covers pair {i-1,i} row-side).
  Col accum: E[:, 2:256] for every row (adds to out[w0+2 .. w0+255]).
  Missing pairs via an extras pass, each end row-side only:
    X1: partner i+256 (d=256); X2: partner i+257/i+255 (d=255, odd low end).

kd-permutation: tile t holds (k, d) for d in {2t, 2t+1}, partition
p = 2k + (d-2t), so the d-sum weight matrix S[p,k] = +-2*(k == p//2) is
identical for every tile (T's columns are permuted host-side).

|z| via relu with per-tile sign s_t (SIGNS): l1 = sum_t 2*sum_d
relu(s_t z) - Gs_j + Gs_i, Gs = sum_t s_t G_t.  Engine variants per tile:
  DVE s=+1: max(M_j - M_i, 0)            -> weight +2 (S2P)
  DVE s=-1: min(M_j - M_i, 0)            -> weight -2 (S2N)
  ACT s=-1: Relu(-(M_j) + M_i)           -> weight +2 (S2P)
-Gs_j is injected into the PSUM chain by a (-I) matmul; +Gs_i rides the
exp per-partition bias.  Values that must cancel exactly on the diagonal
round through bf16.  M tiles t5/t6 stay resident in PSUM so ACT's relu
reads use the cheaper PSUM access path.

Phase 2 packs row pairs (2r, 2r+1) into one PSUM tile (partitions 0:64 /
64:128): the S-matmuls col-tile into concurrent halves, one Exp covers
both rows, E[:,2:256] accumulates into a PSUM O_col via identity-matmul
injects (per-half tile positions (0,0)/(64,64) keep each accumulation
group at a single position - mixing positions in one group hangs the HW).
"""

import os

import numpy as np
import ml_dtypes

import concourse.bass as bass
import concourse.tile as tile
from concourse import mybir
from concourse import bass_utils

B = 512
F = 1024
KD = 1024
NK = 64
DK = 16
N_CORES = 8
NI = B // N_CORES  # 64 local rows
NT = KD // 128  # 8 kd tiles
NF = F // 128  # 8 f chunks
W = 256
JL = NI + W  # 320
NPAIR = NI // 2  # 32

_FP32 = mybir.dt.float32
_BF16 = mybir.dt.bfloat16

AF = mybir.ActivationFunctionType
AO = mybir.AluOpType

# per-tile signs: s_t = -1 for tiles that may run ACT's Relu(-z) variant
SIGNS = [1.0, 1.0, 1.0, 1.0, -1.0, -1.0, -1.0, 1.0]
# ACT tiles: t5/t6 (PSUM-resident M) for both halves, t4 for half 0 only
PSUM_MT = (5, 6)


def engine_for(half, t):
    if t in PSUM_MT:
        return "A"
    if t == 4 and half == 0:
        return "A"
    return "V"


def _split_all_waits(nc):
    """walrus in this env encodes at most 1 sync wait per instruction: hoist
    extra waits onto same-engine NOPs inserted just before the instruction."""
    count = 0
    for fn in nc.m.functions:
        for bb in fn.blocks:
            insts = list(bb.instructions)
            new = []
            changed = False
            for inst in insts:
                si = getattr(inst, "sync_info", None)
                waits = list(si.on_wait) if (si is not None and si.on_wait) else []
                if len(waits) > 1:
                    for w in waits[:-1]:
                        nop = mybir.InstNoOp(name=f"NOPW-{count}", ins=[], outs=[])
                        count += 1
                        nop.engine = inst.engine
                        nop.sync_info = mybir.SyncInfo(on_wait=[w], on_update=[])
                        nc.register_instruction(nop, overwrite=True)
                        new.append(nop)
                    si.on_wait = [waits[-1]]
                    changed = True
                new.append(inst)
            if changed:
                bb.instructions[:] = new


def _patch_drain_wait_limit():
    if getattr(tile.TileContext, "_wait_split_patched", False):
        return
    orig = tile.TileContext.schedule_and_allocate

    def schedule_and_allocate(self, *a, **k):
        r = orig(self, *a, **k)
        _split_all_waits(self.nc)
        return r

    tile.TileContext.schedule_and_allocate = schedule_and_allocate
    tile.TileContext._wait_split_patched = True


def build_host_consts():
    S2P = np.zeros((128, NK), dtype=np.float32)
    for p in range(128):
        S2P[p, p // 2] = 2.0
    S2N = -S2P
    Sg = np.zeros((128, NT * NK), dtype=np.float32)
    for p in range(128):
        for t in range(NT):
            Sg[p, t * NK + p // 2] = SIGNS[t]
    S1 = S2P / 2.0
    I64 = np.concatenate([np.eye(NK, dtype=np.float32)] * 2, axis=0)  # [128,64]
    NI64 = -np.eye(NK, dtype=np.float32)  # [64,64]
    perm = np.empty(KD, dtype=np.int64)
    for t in range(NT):
        for p in range(128):
            perm[t * 128 + p] = (p // 2) * DK + 2 * t + (p % 2)
    consts = {"S2P": S2P, "S2N": S2N, "S1": S1, "Sg": Sg, "I64": I64,
              "NI64": NI64}
    return consts, perm


def build_program():
    _patch_drain_wait_limit()
    nc = bass.Bass(
        "TRN2", target_bir_lowering=False, debug=False, num_devices=N_CORES
    )
    xT_d = nc.dram_tensor("xT", [F, JL], _BF16, kind="ExternalInput").ap()
    T_d = nc.dram_tensor("Tp", [F, KD], _BF16, kind="ExternalInput").ap()
    cdefs = [("S2P", [128, NK]), ("S2N", [128, NK]), ("S1", [128, NK]),
             ("Sg", [128, NT * NK]), ("I64", [128, NK]), ("NI64", [NK, NK])]
    call_d = nc.dram_tensor("Call", [128, 832], _FP32, kind="ExternalInput").ap()
    ocol_d = nc.dram_tensor("ocol", [NK, JL], _FP32, kind="ExternalOutput").ap()
    e2_d = nc.dram_tensor("e2", [128, NPAIR * W], _BF16, kind="ExternalOutput").ap()
    e12_d = nc.dram_tensor("e12", [128, NI], _FP32, kind="ExternalOutput").ap()

    with tile.TileContext(nc) as tc:
        with (
            tc.tile_pool(name="tstage", bufs=2) as tstage,
            tc.tile_pool(name="tr", bufs=NF) as tr_pool,
            tc.tile_pool(name="xr", bufs=NF) as xr_pool,
            tc.tile_pool(name="mt", bufs=1) as mt_pool,
            tc.tile_pool(name="consts", bufs=1) as c_pool,
            tc.tile_pool(name="abs", bufs=8) as abs_pool,
            tc.tile_pool(name="outs", bufs=1) as o_pool,
            tc.tile_pool(name="pmm", bufs=2, space="PSUM") as psum_mm,
            tc.tile_pool(name="pmt", bufs=2, space="PSUM") as psum_mt,
            tc.tile_pool(name="ppair", bufs=3, space="PSUM") as psum_pair,
            tc.tile_pool(name="pocol", bufs=1, space="PSUM") as psum_ocol,
        ):
            # ---- loads (bf16, no casts); descriptor issue split across
            # sync+vector engines, one fat descriptor per T chunk ----
            T_r = []
            for f in range(NF):
                tr = tr_pool.tile([128, KD], _BF16, tag="tr")
                eng = nc.sync if f % 2 == 0 else nc.scalar
                eng.dma_start(out=tr, in_=T_d[f * 128 : (f + 1) * 128, :])
                T_r.append(tr)
            x_all = xr_pool.tile([128, NF * JL], _BF16, tag="xr")
            xv = x_all.rearrange("p (f c) -> p f c", f=NF)
            xs = xT_d.rearrange("(f p) c -> p f c", f=NF)
            nc.sync.dma_start(out=xv[:, 0:4, :], in_=xs[:, 0:4, :])
            nc.scalar.dma_start(out=xv[:, 4:NF, :], in_=xs[:, 4:NF, :])
            x_r = [xv[:, f, :] for f in range(NF)]

            cb = {}
            cst = tstage.tile([128, 832], _FP32, tag="cstage")
            nc.sync.dma_start(out=cst, in_=call_d)
            off = 0
            for name, shape in cdefs:
                tl = c_pool.tile(shape, _BF16, tag=name.lower())
                if shape[0] == 128:
                    nc.vector.tensor_copy(tl, cst[:, off : off + shape[1]])
                else:
                    nc.vector.tensor_copy(tl, cst[0 : shape[0], off : off + shape[1]])
                off += shape[1]
                cb[name] = tl
            S2Pb, S2Nb, S1b = cb["S2P"], cb["S2N"], cb["S1"]
            Sgb, I64b, NI64b = cb["Sg"], cb["I64"], cb["NI64"]

            # ---- phase 1: M^T tiles -> mt_all (bf16) + mcols (fp32);
            # tiles t5/t6 stay in PSUM for ACT's relu reads ----
            mt_all = mt_pool.tile([128, NT * JL], _BF16, tag="mt")
            mcols = mt_pool.tile([128, NT * NK], _FP32, tag="mcols")
            mt_ps = {}
            for t in range(NT):
                pool = psum_mt if t in PSUM_MT else psum_mm
                pm = pool.tile([128, JL], _FP32, tag="pm" if t not in PSUM_MT else "pmt")
                for f in range(NF):
                    nc.tensor.matmul(
                        pm,
                        lhsT=T_r[f][:, t * 128 : (t + 1) * 128],
                        rhs=x_r[f],
                        start=(f == 0),
                        stop=(f == NF - 1),
                    )
                if t in PSUM_MT:
                    mt_ps[t] = pm
                nc.scalar.activation(mt_all[:, t * JL : (t + 1) * JL], pm, AF.Copy)
                if t in PSUM_MT:
                    # ACT reads PSUM fp32 directly; bias must match exactly
                    nc.vector.tensor_copy(
                        mcols[:, t * NK : (t + 1) * NK], pm[:, 0:NK]
                    )
                else:
                    # bias rounds through bf16 so the diagonal is exactly 0
                    nc.vector.tensor_copy(
                        mcols[:, t * NK : (t + 1) * NK],
                        mt_all[:, t * JL : t * JL + NK],
                    )

            # ---- phase 1.5: signed G (Gs) and unsigned G (Gall) ----
            mtv = mt_all.rearrange("p (t w) -> p t w", t=NT)
            pgs = psum_mm.tile([NK, JL], _FP32, tag="pm")
            for t in range(NT):
                nc.tensor.matmul(
                    pgs,
                    lhsT=Sgb[:, t * NK : (t + 1) * NK],
                    rhs=mtv[:, t, :],
                    start=(t == 0),
                    stop=(t == NT - 1),
                )
            Gs_sb = c_pool.tile([NK, JL], _BF16, tag="gs")
            nc.scalar.activation(Gs_sb, pgs, AF.Copy)
            pga = psum_mm.tile([NK, JL], _FP32, tag="pm")
            for t in range(NT):
                nc.tensor.matmul(
                    pga,
                    lhsT=S1b,
                    rhs=mtv[:, t, :],
                    start=(t == 0),
                    stop=(t == NT - 1),
                )
            Gall_sb = c_pool.tile([NK, JL], _BF16, tag="gall")
            nc.scalar.activation(Gall_sb, pga, AF.Copy)
            # exp bias for pair r: GBneg[0:64, r] = -Gs[k, 2r],
            # GBneg[64:128, r] = -Gs[k, 2r+1]  (from the bf16-rounded Gs)
            GBneg = c_pool.tile([128, NPAIR], _FP32, tag="gbneg")
            nc.gpsimd.tensor_scalar(
                GBneg[0:NK, :], Gs_sb[:, 0 : NI : 2], -1.0, None, op0=AO.mult
            )
            nc.gpsimd.tensor_scalar(
                GBneg[NK:128, :], Gs_sb[:, 1 : NI : 2], -1.0, None, op0=AO.mult
            )
            Gp2 = c_pool.tile([NK, NK], _BF16, tag="gp2")
            nc.gpsimd.tensor_copy(Gp2[:, 0 : NK : 2], Gall_sb[:, W + 1 : W + NK : 2])
            nc.gpsimd.tensor_copy(Gp2[:, 1 : NK : 2], Gall_sb[:, W : W + NK - 1 : 2])

            # ---- phase 2: 32 row pairs ----
            ocol = psum_ocol.tile([128, JL], _FP32, tag="pocol")
            Zb = c_pool.tile([128, JL], _BF16, tag="zb")
            nc.vector.memset(Zb, 0.0)
            for half in range(2):
                nc.tensor.matmul(
                    ocol[half * NK : (half + 1) * NK, :],
                    lhsT=I64b[half * NK : (half + 1) * NK, :],
                    rhs=Zb[half * NK : (half + 1) * NK, :],
                    start=True,
                    stop=False,
                    skip_group_check=True,
                )

            def relu_tile_op(half, t, out_ap, i, w0):
                col = mcols[:, t * NK + i : t * NK + i + 1]
                if engine_for(half, t) == "A":
                    in_ap = (
                        mt_ps[t][:, w0 : w0 + W]
                        if t in PSUM_MT
                        else mt_all[:, t * JL + w0 : t * JL + w0 + W]
                    )
                    nc.scalar.activation(out_ap, in_ap, AF.Relu, bias=col, scale=-1.0)
                    return S2Pb
                in_ap = mt_all[:, t * JL + w0 : t * JL + w0 + W]
                if SIGNS[t] > 0:
                    nc.vector.tensor_scalar(
                        out_ap, in_ap, col, 0.0, op0=AO.subtract, op1=AO.max
                    )
                    return S2Pb
                nc.vector.tensor_scalar(
                    out_ap, in_ap, col, 0.0, op0=AO.subtract, op1=AO.min
                )
                return S2Nb

            E2big = mt_pool.tile([128, NPAIR * W], _BF16, tag="e2big")
            for r in range(NPAIR):
                w0 = 2 * r
                ab0 = abs_pool.tile([128, NT * W], _BF16, tag="abs")
                ab1 = abs_pool.tile([128, NT * W], _BF16, tag="abs")
                ab = (ab0, ab1)
                wts = [[None] * NT, [None] * NT]
                for half in range(2):
                    i = 2 * r + half
                    for t in range(NT):
                        wts[half][t] = relu_tile_op(
                            half, t, ab[half][:, t * W : (t + 1) * W], i, w0
                        )
                pp = psum_pair.tile([128, W], _FP32, tag="ppair")
                for t in range(NT):
                    for half in range(2):
                        nc.tensor.matmul(
                            pp[half * NK : (half + 1) * NK, :],
                            lhsT=wts[half][t],
                            rhs=ab[half][:, t * W : (t + 1) * W],
                            start=(t == 0),
                            stop=False,
                            skip_group_check=True,
                        )
                for half in range(2):
                    nc.tensor.matmul(
                        pp[half * NK : (half + 1) * NK, :],
                        lhsT=NI64b,
                        rhs=Gs_sb[:, w0 : w0 + W],
                        start=False,
                        stop=True,
                        skip_group_check=True,
                    )
                E2 = E2big[:, r * W : (r + 1) * W]
                nc.scalar.activation(
                    E2, pp, AF.Exp, scale=-1.0, bias=GBneg[:, r : r + 1]
                )
                for half in range(2):
                    nc.tensor.matmul(
                        ocol[half * NK : (half + 1) * NK, w0 + 2 : w0 + W],
                        lhsT=I64b[half * NK : (half + 1) * NK, :],
                        rhs=E2[half * NK : (half + 1) * NK, 2:W],
                        start=False,
                        stop=(r == NPAIR - 1),
                        skip_group_check=True,
                    )
                if r % 8 == 7:
                    nc.sync.dma_start(
                        out=e2_d[:, (r - 7) * W : (r + 1) * W],
                        in_=E2big[:, (r - 7) * W : (r + 1) * W],
                    )

            # ---- extras: d=256 (X1) and odd-d=255 (X2) pairs ----
            X1 = abs_pool.tile([128, NT * NK], _BF16, tag="x1")
            X2 = abs_pool.tile([128, NT * NK], _BF16, tag="x2")
            x1v = X1.rearrange("p (t w) -> p t w", t=NT)
            x2v = X2.rearrange("p (t w) -> p t w", t=NT)
            nc.vector.tensor_tensor(
                x1v, mtv[:, :, W : W + NK], mtv[:, :, 0:NK], op=AO.subtract
            )
            nc.vector.tensor_tensor(
                x2v[:, :, 0:NK:2],
                mtv[:, :, W + 1 : W + NK : 2],
                mtv[:, :, 0:NK:2],
                op=AO.subtract,
            )
            nc.vector.tensor_tensor(
                x2v[:, :, 1:NK:2],
                mtv[:, :, W : W + NK - 1 : 2],
                mtv[:, :, 1:NK:2],
                op=AO.subtract,
            )
            nc.vector.tensor_scalar(X1, X1, 0.0, None, op0=AO.max)
            nc.vector.tensor_scalar(X2, X2, 0.0, None, op0=AO.max)
            px = psum_pair.tile([128, NK], _FP32, tag="ppair")
            for t in range(NT):
                for half, X in enumerate((X1, X2)):
                    nc.tensor.matmul(
                        px[half * NK : (half + 1) * NK, :],
                        lhsT=S2Pb,
                        rhs=X[:, t * NK : (t + 1) * NK],
                        start=(t == 0),
                        stop=False,
                        skip_group_check=True,
                    )
            # l1x = S2-sum - Gall_partner + Gall_i
            for half, gpart in enumerate((Gall_sb[:, W : W + NK], Gp2[:, :])):
                nc.tensor.matmul(
                    px[half * NK : (half + 1) * NK, :],
                    lhsT=NI64b,
                    rhs=gpart,
                    start=False,
                    stop=False,
                    skip_group_check=True,
                )
            for half in range(2):
                nc.tensor.matmul(
                    px[half * NK : (half + 1) * NK, :],
                    lhsT=I64b[0:NK, :],
                    rhs=Gall_sb[:, 0:NK],
                    start=False,
                    stop=True,
                    skip_group_check=True,
                )
            E12 = o_pool.tile([128, NI], _FP32, tag="e12")
            nc.scalar.activation(E12, px, AF.Exp, scale=-1.0)


            ocol_sb = o_pool.tile([NK, JL], _FP32, tag="ocolsb")
            nc.scalar.activation(ocol_sb, ocol[0:NK, :], AF.Copy)
            nc.vector.tensor_tensor(
                ocol_sb, ocol_sb, ocol[NK:128, :], op=AO.add
            )
            nc.sync.dma_start(out=e12_d, in_=E12)
            nc.sync.dma_start(out=ocol_d, in_=ocol_sb)
    return nc


_CACHED = {}


def _get_program():
    if "nc" not in _CACHED:
        _CACHED["nc"] = build_program()
        _CACHED["consts"] = build_host_consts()
    return _CACHED["nc"], _CACHED["consts"]


def make_in_maps(x: np.ndarray, T: np.ndarray, consts, perm):
    bf16 = ml_dtypes.bfloat16
    xT = np.ascontiguousarray(x.T.astype(np.float32, copy=False))
    Tp = np.ascontiguousarray(
        T.astype(np.float32, copy=False)[:, perm].astype(bf16)
    )
    Call = np.zeros((128, 832), dtype=np.float32)
    off = 0
    for name in ("S2P", "S2N", "S1", "Sg", "I64", "NI64"):
        arr = consts[name]
        Call[0 : arr.shape[0], off : off + arr.shape[1]] = arr
        off += arr.shape[1]
    in_maps = []
    for c in range(N_CORES):
        xTc = np.ascontiguousarray(
            np.roll(xT, -NI * c, axis=1)[:, :JL].astype(bf16)
        )
        in_maps.append({"xT": xTc, "Tp": Tp, "Call": Call})
    return in_maps


def assemble(results) -> np.ndarray:
    out = np.zeros((B, NK), dtype=np.float64)
    for c in range(N_CORES):
        E2 = np.asarray(results[c]["e2"]).astype(np.float32)  # [128, NPAIR*W]
        E12 = results[c]["e12"]  # [128, NI]
        C = results[c]["ocol"]  # [NK, JL]
        base = NI * c
        R = E2.reshape(128, NPAIR, W).sum(axis=2)  # [128, NPAIR]
        for half in range(2):
            rows = base + 2 * np.arange(NPAIR) + half
            out[rows, :] += R[half * NK : (half + 1) * NK, :].T
        out[base : base + NI, :] += E12[0:NK, :].T
        out[base : base + NI, :] += E12[NK:128, :].T
        Cfull = np.zeros((B, NK), dtype=np.float64)
        Cfull[:JL] = C.T
        out += np.roll(Cfull, base, axis=0)
    out -= 1.0  # diagonal exp(0) included in row sums
    return out.astype(np.float32)


def run(x: np.ndarray, T: np.ndarray, trace: bool = False):
    nc, (consts, perm) = _get_program()
    in_maps = make_in_maps(x, T, consts, perm)
    res = bass_utils.run_bass_kernel_spmd(
        nc, in_maps, core_ids=list(range(N_CORES)), trace=trace
    )
    return assemble(res.results), res


def kernel(x: np.ndarray, T: np.ndarray) -> np.ndarray:
    out, _ = run(x, T)
    return out
